# revision 12
# baseline (speedup 1.0000x reference)
"""3-layer GAT (DGL-style GATConv) on 8 Trainium2 NeuronCores via Bass/Tile.

v5: dst-sharded "rounds" formulation, fp8 tables, feature-major aggregation,
chunked AllGather overlapped with block compute.

- Each core owns SH=N/8 dst nodes, sorted by in-degree (desc), lane-per-node:
  node q = b*128+p. Edge r of node q occupies round r. R_b = max in-degree in
  block b over all cores (SPMD).
- Table row (per node, 256B fp8): [f(128 fp8) | el(4 bf16 = 8 bytes) | pad].
  fp8 e4m3 quantization of the projected features passes rel-err 6e-3 in
  simulation (tolerance 2e-2). el is bit-packed bf16 inside the fp8 row.
- Table layout is CHUNK-major: blocks are grouped into NCHUNK chunks; rows of
  chunk k from all cores are contiguous, so the AllGather runs per-chunk and
  overlaps the previous layer's block processing.
- Per edge: e = el[src] + er[dst] (er resident, pre-expanded per round);
  a = max(exp(e), exp(0.2e)) = exp(leaky_relu(e)). alpha = a / z computed
  per block BEFORE aggregation (a whole block's scores complete with its
  gather group), so no divide-by-z epilogue is needed.
- Aggregation: gv = alpha*f on DVE, then per round matmul(lhsT=gv_r,
  rhs=identity) accumulates gv^T in PSUM -> FEATURE-major [f, q] aggregate.
  Epilogue per block is a single ACT relu (PSUM->SBUF, bf16) giving hT-block,
  which is directly the lhsT for the next layer's projection matmul
  (out[q, 136] = [rows | el | er] node-major, via rhs = [W | W@al | W@ar]).
- Layer 2 factorized: aggregate alpha-weighted h per head (4x128), project
  each head's aggregate with W2_h, max-pool feature-major with relu folded
  into max-with-zero-init. Head: AllReduce(max), fc + softmax (replicated).
"""
import numpy as np
import ml_dtypes

BF16 = ml_dtypes.bfloat16
FP8 = ml_dtypes.float8_e4m3fn
P = 128
NC = 8
ROWW = 256   # table row bytes (fp8 elems): [f 128 | el 8 bytes | pad]
MAXG = 36    # max rounds per gather group
RC = 10      # rounds per gv chunk (SBUF working-tile size)
NCHUNK = 4   # AllGather chunks per layer


def _ceil(a, b):
    return -(-a // b)


def _wrap16(seq):
    """dma_gather index layout: [128, n/16] int16, idx i at [i%16, i//16], replicated."""
    n = seq.shape[0]
    assert n % 16 == 0
    w = seq.reshape(n // 16, 16).T.astype(np.int16)  # [16, n/16]
    return np.tile(w, (8, 1))  # [128, n/16]


def _preprocess(src, dst, N, E):
    """Degree-sort nodes per core, chunk layout, per-(lane,round) gather idx."""
    SH = N // NC
    NBLK = _ceil(SH, P)

    deg = np.bincount(dst, minlength=N)
    orders = []
    pos = np.empty(N, np.int64)
    for c in range(NC):
        dl = deg[c * SH:(c + 1) * SH]
        order = np.argsort(-dl, kind="stable")
        orders.append(order)
        pos[c * SH + order] = np.arange(SH)
    R_b = np.zeros(NBLK, np.int64)
    for c in range(NC):
        ds = deg[c * SH:(c + 1) * SH][orders[c]]
        for b in range(NBLK):
            lo = b * P
            if lo < SH:
                R_b[b] = max(R_b[b], int(ds[lo:min(lo + P, SH)].max()))
    R_b = np.maximum(R_b, 1)
    TOT_R = int(R_b.sum())

    # ---- chunk layout: NCHUNK contiguous block ranges, last one smallest ----
    nch = min(NCHUNK, NBLK)
    # boundaries by cumulative R (compute time), biased so last chunk is small
    targets = [TOT_R * (k + 1) / nch for k in range(nch - 1)]
    cum = np.cumsum(R_b)
    bounds = []
    for t in targets:
        bounds.append(int(np.searchsorted(cum, t) + 1))
    bounds = sorted(set(b for b in bounds if 0 < b < NBLK))
    chunk_blocks = []
    prev = 0
    for b in bounds:
        chunk_blocks.append(list(range(prev, b)))
        prev = b
    chunk_blocks.append(list(range(prev, NBLK)))
    nch = len(chunk_blocks)
    # chunk k covers pos range [start_k, end_k); last chunk ends at SH+1 (poison)
    ch_start = [blks[0] * P for blks in chunk_blocks]
    ch_end = [min(blks[-1] * P + P, SH) for blks in chunk_blocks]
    ch_end[-1] = SH + 1   # poison row rides in the last chunk
    ch_len = [e - s for s, e in zip(ch_start, ch_end)]
    ch_off = np.concatenate(([0], np.cumsum(ch_len)))  # row offsets (per core)
    NTAB1 = SH + 1
    assert ch_off[-1] == NTAB1

    def table_row(core, q):
        """Row index in the chunk-major gathered table for (core, pos q)."""
        k = np.searchsorted(np.array(ch_end), q, side="right")
        return NC * ch_off[k] + core * ch_len[k] + (q - ch_start[k])

    # vectorized table_row for arrays
    ch_end_a = np.array(ch_end)

    def table_row_v(core_a, q_a):
        k = np.searchsorted(ch_end_a, q_a, side="right")
        k = np.minimum(k, nch - 1)
        # fix: q < ch_end[k] required; searchsorted 'right' on end gives first
        # k with end > q
        return (NC * ch_off[k] + core_a * np.array(ch_len)[k]
                + (q_a - np.array(ch_start)[k]))

    PAD_ROW = int(table_row(0, SH))

    # ---- gather groups ----
    cap = max(MAXG, int(R_b.max()))
    groups = []
    cur = []
    s = 0
    for b in range(NBLK):
        if cur and s + R_b[b] > cap:
            groups.append(cur)
            cur = []
            s = 0
        cur.append(b)
        s += R_b[b]
    if cur:
        groups.append(cur)
    grp_R = [int(sum(R_b[b] for b in g)) for g in groups]

    src_row = table_row_v(src // SH, pos[src])

    order_edges = np.argsort(dst, kind="stable")
    ss_row = src_row[order_edges]
    dd = dst[order_edges]
    q_of = pos[dd]
    core_of = dd // SH

    core_arrays = []
    for c in range(NC):
        m = core_of == c
        qs = q_of[m]
        rows = ss_row[m]
        o2 = np.argsort(qs, kind="stable")
        qs = qs[o2]
        rows = rows[o2]
        rr = np.arange(qs.shape[0]) - np.concatenate(
            ([0], np.cumsum(np.bincount(qs, minlength=SH))))[qs]
        idx2d = np.full((SH, int(R_b.max())), PAD_ROW, np.int64)
        idx2d[qs, rr] = rows
        fidx = np.zeros((P, TOT_R * 8), np.int16)
        off = 0
        for g, blks in enumerate(groups):
            seq = []
            for b in blks:
                lanes = np.arange(b * P, min((b + 1) * P, SH))
                blkidx = np.full((P, R_b[b]), PAD_ROW, np.int64)
                blkidx[:lanes.shape[0], :] = idx2d[lanes, :R_b[b]]
                seq.append(blkidx.T.reshape(-1))  # round-major, then lane
            seq = np.concatenate(seq)
            fidx[:, off * 8:(off + seq.shape[0] // P) * 8] = _wrap16(seq)
            off += seq.shape[0] // P
        core_arrays.append({"fidx": fidx})

    sched = {
        "SH": SH, "NBLK": NBLK, "R_b": [int(x) for x in R_b], "TOT_R": TOT_R,
        "groups": groups, "grp_R": grp_R, "NTAB1": NTAB1,
        "chunk_blocks": chunk_blocks, "ch_start": ch_start, "ch_end": ch_end,
        "ch_len": ch_len, "ch_off": [int(x) for x in ch_off],
    }
    return sched, core_arrays


def _build_program(sched, FIN, phase=6):
    import concourse.bacc as bacc
    import concourse.mybir as mybir
    import concourse.tile as tile
    from concourse.masks import make_identity

    dt = mybir.dt
    SH, NBLK, R_b = sched["SH"], sched["NBLK"], sched["R_b"]
    TOT_R, groups, grp_R = sched["TOT_R"], sched["groups"], sched["grp_R"]
    NTAB1 = sched["NTAB1"]
    chunk_blocks = sched["chunk_blocks"]
    ch_start, ch_end = sched["ch_start"], sched["ch_end"]
    ch_len, ch_off = sched["ch_len"], sched["ch_off"]
    PBLK = _ceil(NTAB1, P)
    MAXGR = max(grp_R)
    AF = mybir.ActivationFunctionType
    OP = mybir.AluOpType
    FP8DT = dt.float8e4

    # block -> chunk index; chunk k fires after its last block's flush
    blk_chunk = {}
    for k, blks in enumerate(chunk_blocks):
        for b in blks:
            blk_chunk[b] = k
    chunk_last_blk = {k: blks[-1] for k, blks in enumerate(chunk_blocks)}

    nc = bacc.Bacc("TRN2", target_bir_lowering=False, debug=False, num_devices=NC,
                   num_swdge_queues=4)

    xT_in = nc.declare_dram_parameter("xT", [FIN, SH], dt.float32, isOutput=False)
    fidx_in = nc.declare_dram_parameter("fidx", [P, TOT_R * 8], dt.int16, isOutput=False)
    W0w_in = nc.declare_dram_parameter("W0w", [FIN, 136], dt.float32, isOutput=False)
    W1w_in = nc.declare_dram_parameter("W1w", [P, 136], dt.bfloat16, isOutput=False)
    W2id_in = nc.declare_dram_parameter("W2id", [P, 136], dt.bfloat16, isOutput=False)
    W2bf_in = nc.declare_dram_parameter("W2bf", [P, 4 * P], dt.bfloat16, isOutput=False)
    fcw_in = nc.declare_dram_parameter("fcw", [P, 4 * 8], dt.float32, isOutput=False)
    fcb_in = nc.declare_dram_parameter("fcb", [1, 8], dt.float32, isOutput=False)
    out_ext = nc.declare_dram_parameter("out", [1, 8], dt.float32, isOutput=True)
    dbg_ext = nc.declare_dram_parameter("dbg", [P, 512], dt.float32, isOutput=True)

    def dram(name, shape, dtype, shared=False):
        return nc.dram_tensor(name, shape, dtype,
                              addr_space="Shared" if shared else "Local")

    fsh = [dram(f"fsh{l}", [PBLK * P, ROWW], FP8DT) for l in range(3)]
    ftab = [dram(f"ftab{l}", [NC * NTAB1, ROWW], FP8DT, shared=True)
            for l in range(3)]
    pmax_in = dram("pmax_in", [P, 4], dt.float32)
    pmax_out = dram("pmax_out", [P, 4], dt.float32, shared=True)
    rg = [list(range(NC))]

    with tile.TileContext(nc) as tc:
        with (
            tc.tile_pool(name="const", bufs=1) as cp,
            tc.tile_pool(name="pers", bufs=1) as pers,
            tc.tile_pool(name="gath", bufs=4) as gp,
            tc.tile_pool(name="wk", bufs=6) as wk,
            tc.tile_pool(name="gv", bufs=3) as gvp,
            tc.tile_pool(name="ep", bufs=3) as ep,
            tc.tile_pool(name="psum", bufs=4, space="PSUM") as pp,
            tc.tile_pool(name="psacc", bufs=2, space="PSUM") as pacc,
        ):
            f32, bf16 = dt.float32, dt.bfloat16

            def load_const(name, src_ap, shape, dtype):
                t = cp.tile(shape, dtype, tag=name)
                nc.sync.dma_start(out=t[:], in_=src_ap)
                return t

            fidx_sb = load_const("fidx", fidx_in[:], [P, TOT_R * 8], dt.int16)
            W0w_sb = load_const("W0w", W0w_in[:], [FIN, 136], f32)
            W1w_sb = load_const("W1w", W1w_in[:], [P, 136], bf16)
            W2id_sb = load_const("W2id", W2id_in[:], [P, 136], bf16)
            W2bf_sb = load_const("W2bf", W2bf_in[:], [P, 4 * P], bf16)
            fcw_sb = load_const("fcw", fcw_in[:], [P, 4 * 8], f32)
            fcb_sb = load_const("fcb", fcb_in[:], [1, 8], f32)
            identf = cp.tile([P, P], f32, tag="identf")
            make_identity(nc, identf[:])
            identbf = cp.tile([P, P], bf16, tag="identbf")
            nc.vector.tensor_copy(out=identbf[:], in_=identf[:])

            ersb = pers.tile([P, NBLK * 4], f32, tag="ersb")
            erexp = pers.tile([P, TOT_R * 4], f32, tag="erexp")
            fbf = pers.tile([P, PBLK * ROWW], FP8DT, tag="fbf")
            acc_maxT = pers.tile([P, 4 * P], f32, tag="accmaxT")

            # round offset of each block within the global round list
            blk_roff = {}
            roff = 0
            for g, blks in enumerate(groups):
                for b in blks:
                    blk_roff[b] = roff
                    roff += R_b[b]

            def table_block(l, b, psum_row, nv):
                """Write table rows + er for block b from node-major psum
                [nv, 136] = [rows fp8 | el bf16 | er f32-source]."""
                co = b * ROWW
                nc.vector.tensor_copy(out=fbf[:nv, co:co + 128],
                                      in_=psum_row[:nv, 0:128])
                nc.vector.tensor_copy(
                    out=fbf[:nv, co + 128:co + 136].bitcast(bf16),
                    in_=psum_row[:nv, 128:132])
                nc.vector.tensor_copy(out=ersb[:nv, b * 4:(b + 1) * 4],
                                      in_=psum_row[:nv, 132:136])
                # er expansion for this block's rounds
                R = R_b[b]
                ro = blk_roff[b]
                nc.vector.tensor_copy(
                    out=erexp[:, ro * 4:(ro + R) * 4]
                        .rearrange("p (r h) -> p r h", h=4),
                    in_=ersb[:, b * 4:(b + 1) * 4].unsqueeze(1)
                        .to_broadcast([P, R, 4]))

            def poison_init():
                """Poison the last table block (all 128 partitions; the valid
                lanes are overwritten later by table_block): f=0, el=-1000.
                Covers the poison row (pos SH) and any junk lanes beyond it."""
                lb = SH // P
                co = lb * ROWW
                nc.gpsimd.memset(fbf[:, co:co + 128], 0.0)
                # -100 (not -1000): exp(0.2*-100) = 2e-9 stays NONZERO in bf16,
                # so all-poison lanes keep z > 0 and alpha = a/z finite (else
                # 0 * inf = NaN poisons the whole block via the lane-sum mm).
                nc.gpsimd.memset(fbf[:, co + 128:co + 136].bitcast(bf16),
                                 -100.0)

            def fire_chunk(l, k):
                """DMA chunk k of fbf to fsh[l] and AllGather it into ftab[l]."""
                s, e = ch_start[k], ch_end[k]
                b0 = chunk_blocks[k][0]
                # cover through the poison block on the last chunk
                nb = (PBLK - b0) if k == len(chunk_blocks) - 1 \
                    else len(chunk_blocks[k])
                nc.sync.dma_start(
                    out=fsh[l][b0 * P:(b0 + nb) * P, :]
                        .rearrange("(b p) f -> p b f", p=P),
                    in_=fbf[:, b0 * ROWW:(b0 + nb) * ROWW]
                        .rearrange("p (b w) -> p b w", w=ROWW))
                nc.gpsimd.collective_compute(
                    "AllGather", OP.bypass,
                    ins=[fsh[l][s:e, :]],
                    outs=[ftab[l][NC * ch_off[k]:NC * ch_off[k + 1], :]],
                    replica_groups=rg)

            def prep0():
                """Layer-0 table from xT via W0w, block by block, chunked AG."""
                hT0 = pers.tile([P, SH], f32, tag="hT0")
                nc.sync.dma_start(out=hT0[:FIN, :], in_=xT_in[:])
                poison_init()
                for k, blks in enumerate(chunk_blocks):
                    for b in blks:
                        nv = min(P, SH - b * P)
                        pr = pp.tile([P, 136], f32, tag="pp")
                        nc.tensor.matmul(out=pr[:nv, :],
                                         lhsT=hT0[:, b * P:b * P + nv],
                                         rhs=W0w_sb[:], start=True, stop=True)
                        table_block(0, b, pr, nv)
                    fire_chunk(0, k)

            def flush01(l, b, ps):
                """Layer 0/1 block epilogue: relu -> hT-block -> next table."""
                nv = min(P, SH - b * P)
                hTb = ep.tile([P, P], bf16, tag="hTb")
                nc.scalar.activation(out=hTb[:, :nv], in_=ps[:, :nv],
                                     func=AF.Relu)
                rhs = W1w_sb if l == 0 else W2id_sb
                pr = pp.tile([P, 136], f32, tag="pp")
                nc.tensor.matmul(out=pr[:nv, :], lhsT=hTb[:, :nv],
                                 rhs=rhs[:], start=True, stop=True)
                table_block(l + 1, b, pr, nv)
                k = blk_chunk[b]
                if chunk_last_blk[k] == b:
                    fire_chunk(l + 1, k)

            def flush2(b, ps):
                """Layer 2 block epilogue: per-head W2 projection + max-pool
                (relu folded into max against zero-initialized acc)."""
                nv = min(P, SH - b * P)
                for h in range(4):
                    aggT = ep.tile([P, P], bf16, tag="aggT")
                    nc.vector.tensor_copy(out=aggT[:, :nv],
                                          in_=ps[:, h * P:h * P + nv])
                    o2 = pp.tile([P, P], f32, tag="pp")
                    nc.tensor.matmul(out=o2[:, :nv],
                                     lhsT=W2bf_sb[:, h * P:(h + 1) * P],
                                     rhs=aggT[:, :nv], start=True, stop=True)
                    nc.vector.tensor_tensor(
                        out=acc_maxT[:, h * P:(h + 1) * P][:, :nv],
                        in0=acc_maxT[:, h * P:(h + 1) * P][:, :nv],
                        in1=o2[:, :nv], op=OP.max)

            def layer_main(l):
                fw = 512 if l == 2 else P
                off_all = 0
                pending = None
                if l < 2:
                    poison_init()   # for table l+1, built during this layer

                def flush():
                    nonlocal pending
                    if pending is not None:
                        pb, pps = pending
                        if l < 2:
                            flush01(l, pb, pps)
                        else:
                            flush2(pb, pps)
                        pending = None

                for g, blks in enumerate(groups):
                    gR = grp_R[g]
                    o8 = off_all * 8
                    # emit the previous group's last epilogue BEFORE the gather
                    # instructions (sync-pass convoy avoidance)
                    flush()
                    fg = gp.tile([P, MAXGR, ROWW], FP8DT, tag="fg")
                    if gR >= 16:
                        cuts = [0, gR // 4, gR // 2, (3 * gR) // 4, gR]
                    elif gR >= 12:
                        cuts = [0, gR // 3, (2 * gR) // 3, gR]
                    elif gR >= 8:
                        cuts = [0, gR // 2, gR]
                    else:
                        cuts = [0, gR]
                    for j in range(len(cuts) - 1):
                        r0, r1 = cuts[j], cuts[j + 1]
                        nc.gpsimd.dma_gather(
                            out_ap=fg[:, r0:r1, :], in_ap=ftab[l][:],
                            idxs_ap=fidx_sb[:, o8 + r0 * 8:o8 + r1 * 8],
                            num_idxs=(r1 - r0) * P, num_idxs_reg=(r1 - r0) * P,
                            elem_size=ROWW, single_packet=False,
                            queue_num=(3 * g + j) % 4)
                    # group-wide attention scores
                    e4g = wk.tile([P, MAXGR * 4], f32, tag="e4g")
                    nc.vector.tensor_tensor(
                        out=e4g[:, :gR * 4].rearrange("p (r h) -> p r h", h=4),
                        in0=fg[:, :gR, 128:136].bitcast(bf16),
                        in1=erexp[:, off_all * 4:(off_all + gR) * 4]
                            .rearrange("p (r h) -> p r h", h=4),
                        op=OP.add)
                    a1 = wk.tile([P, MAXGR * 4], bf16, tag="a1")
                    a2 = wk.tile([P, MAXGR * 4], bf16, tag="a2")
                    nc.scalar.activation(out=a1[:, :gR * 4], in_=e4g[:, :gR * 4],
                                         func=AF.Exp)
                    nc.scalar.activation(out=a2[:, :gR * 4], in_=e4g[:, :gR * 4],
                                         func=AF.Exp, scale=0.2)
                    ag = wk.tile([P, MAXGR * 4], bf16, tag="ag")
                    nc.vector.tensor_tensor(out=ag[:, :gR * 4], in0=a1[:, :gR * 4],
                                            in1=a2[:, :gR * 4], op=OP.max)
                    roff = 0
                    for b in blks:
                        R = R_b[b]
                        ab = ag[:, roff * 4:(roff + R) * 4]
                        # z, zi, alpha (block scores complete with the group)
                        z4 = wk.tile([P, 4], f32, tag="z4")
                        nc.vector.reduce_sum(
                            out=z4[:],
                            in_=ab.rearrange("p (r h) -> p h r", h=4),
                            axis=mybir.AxisListType.X)
                        zi = wk.tile([P, 4], f32, tag="zi")
                        nc.vector.reciprocal(out=zi[:], in_=z4[:])
                        alpha = wk.tile([P, max(R_b) * 4], bf16, tag="alpha")
                        nc.vector.tensor_tensor(
                            out=alpha[:, :R * 4].rearrange("p (r h) -> p r h", h=4),
                            in0=ab.rearrange("p (r h) -> p r h", h=4),
                            in1=zi[:].unsqueeze(1).to_broadcast([P, R, 4]),
                            op=OP.mult)
                        ps = pacc.tile([P, fw], f32, tag="ps")
                        for r0 in range(0, R, RC):
                            rc = min(RC, R - r0)
                            fgb = fg[:, roff + r0:roff + r0 + rc, :]
                            gv = gvp.tile([P, RC * fw], bf16, tag="gv")
                            if l < 2:
                                nc.vector.tensor_tensor(
                                    out=gv[:, :rc * P]
                                        .rearrange("p (r h c) -> p r h c",
                                                   h=4, c=32),
                                    in0=fgb[:, :, 0:128]
                                        .rearrange("p r (h c) -> p r h c", c=32),
                                    in1=alpha[:, (r0) * 4:(r0 + rc) * 4]
                                        .rearrange("p (r h) -> p r h", h=4)
                                        .unsqueeze(-1).to_broadcast([P, rc, 4, 32]),
                                    op=OP.mult)
                                for r in range(rc):
                                    nc.tensor.matmul(
                                        out=ps[:],
                                        lhsT=gv[:, r * P:(r + 1) * P],
                                        rhs=identbf[:],
                                        start=(r0 == 0 and r == 0),
                                        stop=(r0 + rc == R and r == rc - 1))
                            else:
                                for h in range(4):
                                    nc.vector.tensor_tensor(
                                        out=gv[:, :rc * 512]
                                            .rearrange("p (r f) -> p r f", f=512)
                                            [:, :, h * P:(h + 1) * P],
                                        in0=fgb[:, :, 0:128],
                                        in1=alpha[:, r0 * 4:(r0 + rc) * 4]
                                            .rearrange("p (r h) -> p r h", h=4)
                                            [:, :, h:h + 1]
                                            .to_broadcast([P, rc, P]),
                                        op=OP.mult)
                                for r in range(rc):
                                    for h in range(4):
                                        # start clears has_written for the WHOLE
                                        # bank: only the very first matmul may
                                        # set it; heads 1-3 overwrite-on-clear.
                                        nc.tensor.matmul(
                                            out=ps[:, h * P:(h + 1) * P],
                                            lhsT=gv[:, r * 512 + h * P:
                                                    r * 512 + (h + 1) * P],
                                            rhs=identbf[:],
                                            start=(r0 == 0 and r == 0 and h == 0),
                                            stop=(r0 + rc == R and r == rc - 1))
                        flush()
                        pending = (b, ps)
                        roff += R
                    off_all += gR
                flush()

            # ================= debug helpers =================
            def dump_dbg(ap_f32_cols):
                dt_ = pers.tile([P, 512], f32, tag="dbgt")
                nc.gpsimd.memset(dt_[:], 0.0)
                for ap, c0, w in ap_f32_cols:
                    nc.vector.tensor_copy(out=dt_[:, c0:c0 + w], in_=ap)
                nc.sync.dma_start(out=dbg_ext[:], in_=dt_[:])
                dd = ep.tile([1, 8], f32, tag="ot")
                nc.gpsimd.memset(dd[:], 0.5)
                nc.sync.dma_start(out=out_ext[:], in_=dd[:])

            def dump_tab(l):
                tb = ep.tile([P, ROWW], FP8DT, tag="dump1")
                nc.sync.dma_start(out=tb[:], in_=ftab[l][0:P, :])
                cols = [(tb[:, :128], 0, 128),
                        (tb[:, 128:136].bitcast(bf16), 128, 4)]
                dump_dbg(cols)

            # ================= run the network =================
            nc.gpsimd.memset(acc_maxT[:], 0.0)
            # junk lanes (>= nv) of ersb are never written by table_block;
            # zero them so junk-lane er is finite (NaN would spread via the
            # lane-sum aggregation matmul).
            nc.gpsimd.memset(ersb[:], 0.0)
            prep0()
            if phase == 0:
                dump_tab(0)
            if phase >= 1:
                layer_main(0)
                if phase == 2:
                    dump_tab(1)
            if phase >= 3:
                layer_main(1)
                if phase == 4:
                    dump_tab(2)
            if phase >= 5:
                layer_main(2)
                if phase == 5:
                    dump_dbg([(acc_maxT[:], 0, 512)])

            def head():
                pooledT = ep.tile([P, 4], f32, tag="pooledT")
                nc.vector.reduce_max(
                    out=pooledT[:],
                    in_=acc_maxT[:].rearrange("p (h q) -> p h q", h=4),
                    axis=mybir.AxisListType.X)
                nc.sync.dma_start(out=pmax_in[:], in_=pooledT[:])
                nc.gpsimd.collective_compute(
                    "AllReduce", OP.max,
                    ins=[pmax_in[:]], outs=[pmax_out[:]], replica_groups=rg)
                pm = ep.tile([P, 4], f32, tag="pm")
                nc.sync.dma_start(out=pm[:], in_=pmax_out[:])
                fcp = pp.tile([1, 8], f32, tag="pp")
                for j in range(4):
                    nc.tensor.matmul(out=fcp[:], lhsT=pm[:, j:j + 1],
                                     rhs=fcw_sb[:, j * 8:(j + 1) * 8],
                                     start=(j == 0), stop=(j == 3))
                lg = ep.tile([1, 8], f32, tag="lg")
                nc.vector.tensor_tensor(out=lg[:], in0=fcp[:], in1=fcb_sb[:], op=OP.add)
                mx = ep.tile([1, 1], f32, tag="mx")
                nc.vector.reduce_max(out=mx[:], in_=lg[:], axis=mybir.AxisListType.X)
                nc.vector.tensor_tensor(out=lg[:], in0=lg[:],
                                        in1=mx[:].to_broadcast([1, 8]), op=OP.subtract)
                ex = ep.tile([1, 8], f32, tag="ex")
                nc.scalar.activation(out=ex[:], in_=lg[:], func=AF.Exp)
                sm = ep.tile([1, 1], f32, tag="sm")
                nc.vector.reduce_sum(out=sm[:], in_=ex[:], axis=mybir.AxisListType.X)
                nc.vector.reciprocal(out=sm[:], in_=sm[:])
                ot = ep.tile([1, 8], f32, tag="ot")
                nc.vector.tensor_tensor(out=ot[:], in0=ex[:],
                                        in1=sm[:].to_broadcast([1, 8]), op=OP.mult)
                nc.sync.dma_start(out=out_ext[:], in_=ot[:])

            if phase >= 6:
                head()

    nc.finalize()
    return nc


def _host_consts(W0, al0, ar0, W1, al1, ar1, W2, al2, ar2, fc_w, fc_b):
    def foldmat(v):
        hh, cc = v.shape
        m = np.zeros((hh * cc, hh), np.float32)
        for h in range(hh):
            m[h * cc:(h + 1) * cc, h] = v[h]
        return m

    def wfold(W, al, ar):
        W = W.astype(np.float64)
        return np.concatenate(
            [W, W @ foldmat(al).astype(np.float64),
             W @ foldmat(ar).astype(np.float64)], axis=1).astype(np.float32)

    W2id = np.concatenate(
        [np.eye(128, dtype=np.float64),
         W2.astype(np.float64) @ foldmat(al2).astype(np.float64),
         W2.astype(np.float64) @ foldmat(ar2).astype(np.float64)],
        axis=1).astype(np.float32)
    fcw = np.ascontiguousarray(
        fc_w.reshape(4, P, 8).transpose(1, 0, 2).reshape(P, 32)).astype(np.float32)
    return {
        "W0w": wfold(np.asarray(W0), al0, ar0).astype(np.float32),
        "W1w": wfold(np.asarray(W1), al1, ar1).astype(BF16),
        "W2id": W2id.astype(BF16),
        "W2bf": np.ascontiguousarray(W2).astype(BF16),
        "fcw": fcw, "fcb": fc_b.reshape(1, 8).astype(np.float32),
    }


_PROG_CACHE = {}


def run_gat(inputs, src, dst, W0, al0, ar0, W1, al1, ar1, W2, al2, ar2, fc_w, fc_b,
            trace=False):
    from concourse.bass_utils import run_bass_kernel_spmd
    inputs = np.asarray(inputs)
    N, FIN = inputs.shape
    E = np.asarray(src).shape[0]
    sched, core_arrays = _preprocess(np.asarray(src).astype(np.int64),
                                     np.asarray(dst).astype(np.int64), N, E)
    import os
    phase = int(os.environ.get("GAT_PHASE", "6"))
    key = (N, E, FIN, tuple(sched["R_b"]), phase)
    if key not in _PROG_CACHE:
        _PROG_CACHE[key] = _build_program(sched, FIN, phase)
    nc = _PROG_CACHE[key]
    consts = _host_consts(np.asarray(W0), np.asarray(al0), np.asarray(ar0),
                          np.asarray(W1), np.asarray(al1), np.asarray(ar1),
                          np.asarray(W2), np.asarray(al2), np.asarray(ar2),
                          np.asarray(fc_w), np.asarray(fc_b))
    SH = sched["SH"]
    deg = np.bincount(np.asarray(dst), minlength=N)
    in_maps = []
    for c in range(NC):
        m = dict(consts)
        m.update(core_arrays[c])
        order = np.argsort(-deg[c * SH:(c + 1) * SH], kind="stable")
        m["xT"] = np.ascontiguousarray(
            inputs[c * SH:(c + 1) * SH, :][order, :].T).astype(np.float32)
        in_maps.append(m)
    res = run_bass_kernel_spmd(nc, in_maps, list(range(NC)), trace=trace)
    out = np.asarray(res.results[0]["out"])
    run_gat.last_dbg = np.asarray(res.results[0].get("dbg")) if "dbg" in res.results[0] else None
    return out, res


def kernel(**inputs):
    out, _ = run_gat(**inputs)
    return out
